# revision 4
# baseline (speedup 1.0000x reference)
# Causal self-attention (GQA, RoPE) on 8 NeuronCores.
#
# Sharding: sequence-parallel. Core c = (batch b = c//4, role r = c%4).
# Each batch's 2048 tokens are split into 8 chunks of 256; role r owns
# chunks {r, 7-r} (zigzag, balances causal work). Each core computes
# QKV for its 512 tokens, AllGathers RoPE'd K^T and ones-augmented V
# within its 4-core batch group, runs causal attention for a uniform
# 24-slot schedule (SPMD needs identical instruction streams; per-core
# causal validity is data: invalid kv tiles contribute zero because
# their V tile incl. the ones column is zeroed), then projects its own
# token rows. No reduction needed after proj.
#
# All matmuls run in float32r (full PE speed at N>=256, ~1e-4 rel err).
# Softmax denominator is the 65th row of the AV matmul (ones column);
# no max subtraction (scores are bounded, fp32 exp is safe).
import sys

sys.path.insert(0, "/opt/trn_rl_repo")
import numpy as np

B, T, C = 2, 2048, 2048
NH, G, HS = 32, 8, 64
QPK = NH // G
NCORES = 8
CHUNK = 256
NCH = T // CHUNK          # 8 chunks per batch
NSLOT_B, NSLOT_A = 16, 8  # uniform kv 128-slots for the two q-chunks
NPREF = 14                # gathered prefix slots resident in SBUF

# Head ordering: q-tile t holds (EVEN_HEADS[t] at partitions 0-63,
# ODD_HEADS[t] at 64-127) so the K-slice partition base (g%2)*64 always
# matches the q-slice base.
EVEN_HEADS = [h for h in range(NH) if (h // QPK) % 2 == 0]
ODD_HEADS = [h for h in range(NH) if (h // QPK) % 2 == 1]


def tok_ids(r):
    a = list(range(r * CHUNK, (r + 1) * CHUNK))
    b = list(range((7 - r) * CHUNK, (8 - r) * CHUNK))
    return np.array(a + b, dtype=np.int64)


def perm_q():
    # reordered q feature j = t*128 + s*64 + d  ->  original attn_w row
    p = np.zeros(NH * HS, dtype=np.int64)
    for t in range(16):
        for s, h in ((0, EVEN_HEADS[t]), (1, ODD_HEADS[t])):
            g, qi = h // QPK, h % QPK
            for d in range(HS):
                p[t * 128 + s * 64 + d] = g * 384 + qi * 64 + d
    return p


def perm_k():
    p = np.zeros(G * HS, dtype=np.int64)
    for g in range(G):
        for d in range(HS):
            p[g * 64 + d] = g * 384 + 256 + d
    return p


def perm_v():
    p = np.zeros(G * HS, dtype=np.int64)
    for g in range(G):
        for d in range(HS):
            p[g * 64 + d] = g * 384 + 320 + d
    return p


def perm_y():
    # y^T row i = t*128 + s*64 + d -> proj_w column h*64+d
    p = np.zeros(NH * HS, dtype=np.int64)
    for t in range(16):
        for s, h in ((0, EVEN_HEADS[t]), (1, ODD_HEADS[t])):
            for d in range(HS):
                p[t * 128 + s * 64 + d] = h * 64 + d
    return p


def head_at(t, s):
    return EVEN_HEADS[t] if s == 0 else ODD_HEADS[t]


def slot_src(s):
    # gathered prefix slot s (kv 128-chunk index s) -> (rank, 256-chunk pos, col128)
    ci = s // 2
    if ci < 4:
        return ci, 0, s % 2
    return 7 - ci, 1, s % 2


def valid_tables(r):
    # validB[s]: qcB (chunk 7-r) prefix slot s valid; validA[s]: qcA (chunk r)
    vB = np.zeros(16, np.float32)
    vA = np.zeros(16, np.float32)
    for s in range(NPREF):
        vB[s] = 1.0 if s <= 13 - 2 * r else 0.0
    for s in range(6):
        vA[s] = 1.0 if s <= 2 * r - 1 else 0.0
    return vA, vB


def host_masks():
    i = np.arange(128)[:, None]
    j = np.arange(256)[None, :]
    m0 = (i <= j).astype(np.float32)
    m1 = (128 + i <= j).astype(np.float32)
    return m0, m1


_PROG = {}


def _build_program():
    if "nc" in _PROG:
        return _PROG
    import concourse.bass as bass
    import concourse.tile as tile
    from concourse import bacc, mybir
    from contextlib import ExitStack

    f32 = mybir.dt.float32
    fr = mybir.dt.float32r
    AF = mybir.ActivationFunctionType

    nc = bacc.Bacc("TRN2", target_bir_lowering=False, debug=False, num_devices=NCORES)

    xT_d = nc.dram_tensor("xT", [C, 512], fr, kind="ExternalInput").ap()
    wqkT_d = nc.dram_tensor("wqkT", [C, 2560], fr, kind="ExternalInput").ap()
    wvT_d = nc.dram_tensor("wvT", [C, 512], fr, kind="ExternalInput").ap()
    pwT_d = nc.dram_tensor("pwT", [C, C], fr, kind="ExternalInput").ap()
    bqk_d = nc.dram_tensor("bqk", [128, 20], f32, kind="ExternalInput").ap()
    bv_d = nc.dram_tensor("bv", [128, 512], f32, kind="ExternalInput").ap()
    pb_d = nc.dram_tensor("pb", [128, C], f32, kind="ExternalInput").ap()
    cos_d = nc.dram_tensor("cosT2", [128, 512], f32, kind="ExternalInput").ap()
    sin_d = nc.dram_tensor("sinT2s", [128, 512], f32, kind="ExternalInput").ap()
    mask_d = nc.dram_tensor("masks", [128, 2, 256], f32, kind="ExternalInput").ap()
    valid_d = nc.dram_tensor("valid", [128, 32], f32, kind="ExternalInput").ap()
    vones_d = nc.dram_tensor("vones", [128, 4, 8], fr, kind="ExternalInput").ap()
    out_d = nc.dram_tensor("out", [512, C], f32, kind="ExternalOutput").ap()

    GROUPS = [[0, 1, 2, 3], [4, 5, 6, 7]]

    with tile.TileContext(nc) as tc:
        with ExitStack() as ctx:
            consts = ctx.enter_context(tc.tile_pool(name="consts", bufs=1))
            qy = ctx.enter_context(tc.tile_pool(name="qy", bufs=1))
            kvloc = ctx.enter_context(tc.tile_pool(name="kvloc", bufs=1))
            dram = ctx.enter_context(tc.tile_pool(name="dram", bufs=1, space="DRAM"))

            cos_s = consts.tile([128, 512], f32)
            sin_s = consts.tile([128, 512], f32)
            bqk_s = consts.tile([128, 20], f32)
            bv_s = consts.tile([128, 512], f32)
            pb_s = consts.tile([128, C], f32)
            mask_s = consts.tile([128, 2, 256], f32)
            valid_s = consts.tile([128, 32], f32)
            nc.sync.dma_start(out=cos_s, in_=cos_d)
            nc.sync.dma_start(out=sin_s, in_=sin_d)
            nc.sync.dma_start(out=bqk_s, in_=bqk_d)
            nc.sync.dma_start(out=bv_s, in_=bv_d)
            nc.sync.dma_start(out=pb_s, in_=pb_d)
            nc.sync.dma_start(out=mask_s, in_=mask_d)
            nc.sync.dma_start(out=valid_s, in_=valid_d)

            qT_s = qy.tile([128, 16, 512], fr)
            yT_s = qy.tile([128, 16, 512], fr)
            kTl_s = kvloc.tile([128, 4, 512], fr)
            vAl_s = kvloc.tile([128, 4, 8, 65], fr)

            k_loc = dram.tile([4, 128, 512], fr)
            v_loc = dram.tile([4, 128, 8, 65], fr)
            k_gat = dram.tile([4, 4, 128, 512], fr)
            v_gat = dram.tile([4, 4, 128, 8, 65], fr)

            def rope_into(dst, ps, bias_col, rp):
                # dst[128, 512] (fp32r) <- rope(ps + bias); the 32-row
                # rotate-half swap is done by SBUF->SBUF DMAs (DVE needs
                # equal base partitions for two SBUF operands).
                tb = rp.tile([128, 512], f32, tag="tb", name="tb")
                nc.scalar.activation(tb, ps, AF.Identity, bias=bias_col)
                t2 = rp.tile([128, 512], f32, tag="t2", name="t2")
                nc.vector.tensor_mul(t2, tb, sin_s)
                tcs = rp.tile([128, 512], f32, tag="tc", name="tcs")
                nc.vector.tensor_mul(tcs, tb, cos_s)
                t2s = rp.tile([128, 512], f32, tag="t2s", name="t2s")
                for b0 in (0, 64):
                    nc.sync.dma_start(out=t2s[b0:b0 + 32, :], in_=t2[b0 + 32:b0 + 64, :])
                    nc.sync.dma_start(out=t2s[b0 + 32:b0 + 64, :], in_=t2[b0:b0 + 32, :])
                nc.vector.tensor_add(dst, t2s, tcs)

            # ---------------- phase 0: QKV projections ----------------
            with ExitStack() as p0:
                xp = p0.enter_context(tc.tile_pool(name="xp", bufs=1))
                wp = p0.enter_context(tc.tile_pool(name="wp", bufs=6))
                rp = p0.enter_context(tc.tile_pool(name="rp", bufs=2))
                ps0 = p0.enter_context(tc.tile_pool(name="ps0", bufs=2, space="PSUM"))

                xT_s = xp.tile([128, 16, 512], fr)
                nc.sync.dma_start(out=xT_s, in_=xT_d.rearrange("(k p) t -> p k t", p=128))

                # K^T tiles
                for kt in range(4):
                    ps = ps0.tile([128, 512], f32, tag="pk", name="ps")
                    for kc in range(16):
                        w = wp.tile([128, 128], fr, tag="w", name="w")
                        nc.sync.dma_start(out=w, in_=wqkT_d[kc * 128:(kc + 1) * 128, 2048 + kt * 128:2048 + (kt + 1) * 128])
                        nc.tensor.matmul(ps, w, xT_s[:, kc, :], start=(kc == 0), stop=(kc == 15))
                    rope_into(kTl_s[:, kt, :], ps, bqk_s[:, 16 + kt:17 + kt], rp)
                nc.sync.dma_start(out=k_loc.rearrange("k p t -> p k t"), in_=kTl_s)
                nc.gpsimd.collective_compute(
                    "AllGather", mybir.AluOpType.bypass, replica_groups=GROUPS,
                    ins=[k_loc.opt()], outs=[k_gat.opt()])

                # V tiles (natural layout, bias, ones column)
                ones_done = False
                psv = [ps0.tile([128, 512], f32, tag="pv", bufs=4, name=f"psv{mt}") for mt in range(4)]
                for kc in range(16):
                    wv = wp.tile([128, 512], fr, tag="wv", bufs=3, name="wv")
                    nc.sync.dma_start(out=wv, in_=wvT_d[kc * 128:(kc + 1) * 128, :])
                    for mt in range(4):
                        nc.tensor.matmul(psv[mt], xT_s[:, kc, mt * 128:(mt + 1) * 128], wv,
                                         start=(kc == 0), stop=(kc == 15))
                nc.sync.dma_start(out=vAl_s[:, :, :, 64:65],
                                  in_=vones_d.rearrange("p c (g o) -> p c g o", o=1))
                for mt in range(4):
                    nc.vector.tensor_add(
                        vAl_s[:, mt, :, 0:64],
                        psv[mt].rearrange("p (g d) -> p g d", g=8),
                        bv_s.rearrange("p (g d) -> p g d", g=8))
                nc.sync.dma_start(out=v_loc.rearrange("c p g d -> p c g d"), in_=vAl_s)
                nc.gpsimd.collective_compute(
                    "AllGather", mybir.AluOpType.bypass, replica_groups=GROUPS,
                    ins=[v_loc.opt()], outs=[v_gat.opt()])

                # Q^T tiles
                for qt in range(16):
                    ps = ps0.tile([128, 512], f32, tag="pk", name="ps")
                    for kc in range(16):
                        w = wp.tile([128, 128], fr, tag="w", name="w")
                        nc.sync.dma_start(out=w, in_=wqkT_d[kc * 128:(kc + 1) * 128, qt * 128:(qt + 1) * 128])
                        nc.tensor.matmul(ps, w, xT_s[:, kc, :], start=(kc == 0), stop=(kc == 15))
                    rope_into(qT_s[:, qt, :], ps, bqk_s[:, qt:qt + 1], rp)

            # ---------------- phase 1: attention ----------------
            with ExitStack() as pa:
                at = pa.enter_context(tc.tile_pool(name="at", bufs=1))
                ep = pa.enter_context(tc.tile_pool(name="ep", bufs=4))
                psA = pa.enter_context(tc.tile_pool(name="psA", bufs=1, space="PSUM"))

                kg_s = at.tile([128, NPREF, 4, 128], fr)
                vg_s = at.tile([128, NPREF, 8, 65], fr)
                for s in range(NPREF):
                    rk, cp, half = slot_src(s)
                    nc.sync.dma_start(out=kg_s[:, s, :, :],
                                      in_=k_gat[rk, :, :, cp * 256 + half * 128: cp * 256 + (half + 1) * 128]
                                      .rearrange("k p t -> p k t"))
                    nc.sync.dma_start(out=vg_s[:, s, :, :], in_=v_gat[rk, cp * 2 + half, :, :, :])
                # stage qcB validity in place
                for s in range(NPREF):
                    nc.vector.tensor_scalar_mul(vg_s[:, s, :, :], vg_s[:, s, :, :].bitcast(f32), valid_s[:, s:s + 1])

                def attend(qc, nslots):
                    for t in range(16):
                        for sh in range(2):
                            h = head_at(t, sh)
                            g = h // QPK
                            kt = g // 2
                            ps_av = psA.tile([65, 256], f32, tag="av", bufs=2, name="ps_av")
                            rhs_q = qT_s[sh * 64:(sh + 1) * 64, t, qc * 256:(qc + 1) * 256]
                            for slot in range(nslots):
                                ps_s = psA.tile([128, 256], f32, tag="s", bufs=4, name="ps_s")
                                if slot < 2:
                                    lhsK = kTl_s[sh * 64:(sh + 1) * 64, kt, qc * 256 + slot * 128: qc * 256 + (slot + 1) * 128]
                                else:
                                    lhsK = kg_s[sh * 64:(sh + 1) * 64, slot - 2, kt, :]
                                nc.tensor.matmul(ps_s, lhsK, rhs_q, start=True, stop=True)
                                e = ep.tile([128, 256], fr, tag="e", name="e")
                                nc.scalar.activation(e, ps_s, AF.Exp, scale=0.125)
                                if slot < 2:
                                    nc.vector.tensor_mul(e, e.bitcast(f32), mask_s[:, slot, :])
                                if slot < 2:
                                    lhsV = vAl_s[:, qc * 2 + slot, g, :]
                                else:
                                    lhsV = vg_s[:, slot - 2, g, :]
                                nc.tensor.matmul(ps_av, lhsV, e, start=(slot == 0), stop=(slot == nslots - 1))
                            r_ = ep.tile([1, 256], f32, tag="r", bufs=3, name="r_")
                            nc.vector.reciprocal(r_, ps_av[64:65, :])
                            rb = ep.tile([64, 256], f32, tag="rb", bufs=3, name="rb")
                            nc.gpsimd.partition_broadcast(rb, r_)
                            nc.vector.tensor_mul(yT_s[sh * 64:(sh + 1) * 64, t, qc * 256:(qc + 1) * 256],
                                                 ps_av[0:64, :], rb)

                attend(1, NSLOT_B)
                # tighten validity for qcA (valid_A subset of valid_B)
                for s in range(6):
                    nc.vector.tensor_scalar_mul(vg_s[:, s, :, :], vg_s[:, s, :, :].bitcast(f32), valid_s[:, 16 + s:17 + s])
                attend(0, NSLOT_A)

            # ---------------- phase 2: output projection ----------------
            with ExitStack() as pp:
                pr = pp.enter_context(tc.tile_pool(name="pr", bufs=4))
                psP = pp.enter_context(tc.tile_pool(name="psP", bufs=8, space="PSUM"))
                for n in range(4):
                    pss = [psP.tile([128, 512], f32, tag="pp", name=f"pss{mt}") for mt in range(4)]
                    for kd in range(16):
                        wpj = pr.tile([128, 512], fr, tag="wpj", name="wpj")
                        nc.sync.dma_start(out=wpj, in_=pwT_d[kd * 128:(kd + 1) * 128, n * 512:(n + 1) * 512])
                        for mt in range(4):
                            nc.tensor.matmul(pss[mt], yT_s[:, kd, mt * 128:(mt + 1) * 128], wpj,
                                             start=(kd == 0), stop=(kd == 15))
                    for mt in range(4):
                        ost = pr.tile([128, 512], f32, tag="ost", name="ost")
                        nc.vector.tensor_add(ost, pss[mt], pb_s[:, n * 512:(n + 1) * 512])
                        nc.sync.dma_start(out=out_d[mt * 128:(mt + 1) * 128, n * 512:(n + 1) * 512], in_=ost)

    nc.compile()
    _PROG["nc"] = nc
    return _PROG


def make_in_maps(x, cos, sin, attn_w, attn_b, proj_w, proj_b):
    x = np.asarray(x, np.float32)
    cos = np.asarray(cos, np.float32)
    sin = np.asarray(sin, np.float32)
    attn_w = np.asarray(attn_w, np.float32)
    attn_b = np.asarray(attn_b, np.float32)
    proj_w = np.asarray(proj_w, np.float32)
    proj_b = np.asarray(proj_b, np.float32)

    pq, pk, pv, py = perm_q(), perm_k(), perm_v(), perm_y()
    pqk = np.concatenate([pq, pk])
    wqkT = np.ascontiguousarray(attn_w[pqk, :].T)          # [2048, 2560]
    wvT = np.ascontiguousarray(attn_w[pv, :].T)            # [2048, 512]
    pwT = np.ascontiguousarray(proj_w.T[py, :])            # [2048, 2048]
    bqk = np.ascontiguousarray(attn_b[pqk].reshape(20, 128).T)   # [128, 20]
    bv = np.tile(attn_b[pv][None, :], (128, 1)).astype(np.float32)
    pb = np.tile(proj_b[None, :], (128, 1)).astype(np.float32)
    m0, m1 = host_masks()
    masks = np.stack([m0, m1], axis=0).transpose(1, 0, 2).copy()  # [128, 2, 256]

    in_maps = []
    for c in range(NCORES):
        b, r = c // 4, c % 4
        ids = tok_ids(r)
        xT = np.ascontiguousarray(x[b, ids, :].T)          # [2048, 512]
        cl = cos[ids, :].T                                  # [64, 512]
        sl = sin[ids, :].T.copy()
        sl[32:] *= -1.0
        cosT2 = np.concatenate([cl, cl], axis=0).astype(np.float32)
        sinT2s = np.concatenate([sl, sl], axis=0).astype(np.float32)
        vA, vB = valid_tables(r)
        valid = np.zeros((128, 32), np.float32)
        valid[:, 0:16] = vB[None, :]
        valid[:, 16:32] = vA[None, :]
        in_maps.append({
            "xT": xT, "wqkT": wqkT, "wvT": wvT, "pwT": pwT,
            "bqk": bqk, "bv": bv, "pb": pb,
            "cosT2": cosT2, "sinT2s": sinT2s, "masks": masks, "valid": valid,
            "vones": np.ones((128, 4, 8), np.float32),
        })
    return in_maps


def assemble_output(results):
    out = np.zeros((B, T, C), np.float32)
    for c in range(NCORES):
        b, r = c // 4, c % 4
        ids = tok_ids(r)
        out[b, ids, :] = results[c]["out"]
    return out


def kernel(**inputs):
    from concourse.bass_utils import run_bass_kernel_spmd

    prog = _build_program()
    in_maps = make_in_maps(**inputs)
    res = run_bass_kernel_spmd(prog["nc"], in_maps, list(range(NCORES)))
    return assemble_output(res.results)


if __name__ == "__main__":
    import reference

    inputs = {k: np.asarray(v) for k, v in reference.setup_inputs().items()}
    expected = np.asarray(reference.reference(**inputs))
    actual = kernel(**inputs)
    err = np.abs(actual - expected).max()
    rel = np.abs(actual - expected).max() / np.abs(expected).max()
    print(f"abs={err:.3e} rel={rel:.3e}")
